# revision 49
# baseline (speedup 1.0000x reference)
"""Multi-head self-attention on 8 Trainium2 NeuronCores.

Problem: B=2, L=2048, E=1024, H=16 heads, D=64 (fp32).
Sharding: 2-way batch x 4-way head-group. Core c handles batch c//4 and
heads 4*(c%4) .. 4*(c%4)+3 (a 256-wide slice of the QKV output dim).
Each core computes a partial output y_c = Attn_c @ W_O[slice]; the host
sums the 4 partials per batch (the "all-reduce" of row-parallel W_O).

v2 design notes (vs the 316us baseline):
 - All inputs host-cast to bf16: input DMA halves (28MB -> 14MB); phase 1
   was DMA-queue-saturated 10..95us in the baseline trace.
 - Phase 1 runs l-outer with [128, 1024] x-tiles so the first matmul
   starts ~3us in (baseline: 40us idle waiting on 1MB gather DMAs).
 - V is projected directly in [l, o] layout (x chunk stationary, W_V
   moving) - no PE transpose pass, no bounce copies.
 - bf16 stationary weights of 128 cols enable FWL (f32r paid a serial
   LDWEIGHTS per matmul).
 - Scores for the two heads of a pair are emitted interleaved at PE row
   offsets 0/64 (K=64): disjoint row-groups execute concurrently in the
   PE array (~2x on the scores stream).
 - Attention loops qt-outer; the output projection for each q-tile is
   emitted in two 4-e-chunk half-lumps at pair boundaries, overlapping
   the W_O matmuls, PSUM->SBUF copies and y DMAs with the next q-tile's
   attention (baseline had a ~50us serial tail). The W_O PSUM tiles
   share the p_o pool's 2-bank tag, claiming the slots in the window
   between a pair's normalize and the next pair's first PV.
 - Scores are emitted 2 stages ahead (st double-buffer allows it) so
   ScalarE's exp stream stays fed across the boundary bubbles; in steady
   state exp runs back-to-back (72 of 87 inter-call gaps are 0 ns).
 - Softmax denominators ride as a ones-column in the PV stationary
   (row 64 of p_o); exp on ScalarE with the 1/sqrt(D) scale folded in.
   The p_o bank is read directly by the normalize mul: a 65-partition
   PSUM->SBUF copy (tried for earlier bank release) corrupts on HW.
"""

import sys

if "/opt/trn_rl_repo" not in sys.path:
    sys.path.insert(0, "/opt/trn_rl_repo")

import numpy as np
import ml_dtypes

B, L, E = 2, 2048, 1024
H, D = 16, 64
OC = 256          # per-core slice of the H*D output dim (4 heads)
HC = OC // D      # heads per core = 4
ECH = E // 128    # 8 e-chunks
LT = L // 512     # 4 l-tiles of 512 (q tiles)
LH = L // 1024    # 2 l-halves (x DMA granularity)
KC = L // 128     # 16 k-chunks
GRP = 3           # (head,kc) units per exp call / score-stage
PAIR_INTERLEAVE = True  # alternate the pair's heads per score matmul

_CACHE = {}


def _build():
    from concourse import bacc, tile, mybir

    f32 = mybir.dt.float32
    bf16 = mybir.dt.bfloat16
    Exp = mybir.ActivationFunctionType.Exp
    Copy = mybir.ActivationFunctionType.Copy

    nc = bacc.Bacc("TRN2", target_bir_lowering=False, debug=False)

    qT = nc.dram_tensor("qT", [E, L], bf16, kind="ExternalInput").ap()
    kT = nc.dram_tensor("kT", [E, L], bf16, kind="ExternalInput").ap()
    vT = nc.dram_tensor("vT", [E, L], bf16, kind="ExternalInput").ap()
    # weights pre-swizzled on host to [128, ECH*OC] / [128, 2*E]
    wq = nc.dram_tensor("wq", [128, ECH * OC], bf16, kind="ExternalInput").ap()
    wk = nc.dram_tensor("wk", [128, ECH * OC], bf16, kind="ExternalInput").ap()
    wv = nc.dram_tensor("wv", [128, ECH * OC], bf16, kind="ExternalInput").ap()
    wo = nc.dram_tensor("wo", [128, 2 * E], bf16, kind="ExternalInput").ap()
    bq = nc.dram_tensor("bq", [128, 2, 1], f32, kind="ExternalInput").ap()
    bk = nc.dram_tensor("bk", [128, 2, 1], f32, kind="ExternalInput").ap()
    # bf16 partials: the host sums 4 per batch in f32; halves the y DMA
    yT = nc.dram_tensor("yT", [E, L], bf16, kind="ExternalOutput").ap()

    qTr = qT.rearrange("(c p) l -> p c l", p=128)   # [128, 8, 2048]
    kTr = kT.rearrange("(c p) l -> p c l", p=128)
    vTr = vT.rearrange("(c p) l -> p c l", p=128)
    wqr = wq.rearrange("p (c o) -> p c o", o=OC)    # [128, 8, 256]
    wkr = wk.rearrange("p (c o) -> p c o", o=OC)
    wvr = wv.rearrange("p (c o) -> p c o", o=OC)
    wor = wo.rearrange("p (c e) -> p c e", e=E)     # [128, 2, 1024]

    with tile.TileContext(nc) as tc:
        with (
            tc.tile_pool(name="w", bufs=1) as wp,
            tc.tile_pool(name="xt", bufs=16) as xp,
            tc.tile_pool(name="qk", bufs=1) as qkp,
            tc.tile_pool(name="vt", bufs=1) as vtp,
            tc.tile_pool(name="et", bufs=5) as ep,
            tc.tile_pool(name="norm", bufs=4) as npl,
            tc.tile_pool(name="yst", bufs=4) as ysp,
        ):
            # ---- weights + biases resident. All input DMAs go on the SP
            # HWDGE ring (the ACT ring's sequencer stream must stay clear:
            # a trigger stalled on a tile-slot semaphore would head-of-line
            # block the attention exp instructions). Each weight DMA is
            # emitted just before the loop that consumes it so the SP FIFO
            # never makes a consumer wait on bytes it doesn't need yet. ----
            twq = wp.tile([128, ECH, OC], bf16, tag="twq")
            twk = wp.tile([128, ECH, OC], bf16, tag="twk")
            twv = wp.tile([128, ECH, OC], bf16, tag="twv")
            two = wp.tile([128, 2, E], bf16, tag="two")
            tbq = wp.tile([128, 2, 1], f32, tag="tbq")
            tbk = wp.tile([128, 2, 1], f32, tag="tbk")

            # ---- persistent activations (all bf16) ----
            qt_t = [qkp.tile([128, L], bf16, tag=f"qt{m}", name=f"qt{m}") for m in range(2)]
            kt_t = [qkp.tile([128, L], bf16, tag=f"kt{m}", name=f"kt{m}") for m in range(2)]
            ot_t = [qkp.tile([128, L], bf16, tag=f"ot{m}", name=f"ot{m}") for m in range(2)]
            # V with a ones column per head: [l, h, d+1]
            v_t = [vtp.tile([128, HC, D + 1], bf16, tag=f"v{i}", name=f"v{i}")
                   for i in range(KC)]

            # ================= phase 1: QKV projections =================
            with tc.tile_pool(name="ps1", bufs=1, space="PSUM") as psp:
                # ---- Q then K: out [o, l] accumulated over 8 e-chunks;
                # 2 live PSUM banks per l-tile ----
                dma_engs = [nc.sync, nc.scalar]
                dma_rr = [0]

                def dma_in(dst, src):
                    dma_engs[dma_rr[0] % 2].dma_start(dst, src)
                    dma_rr[0] += 1

                for ti, (src_r, wt, wtr_src, tb, tb_src, dst, ptag) in enumerate((
                        (qTr, twq, wqr, tbq, bq, qt_t, "pqk"),
                        (kTr, twk, wkr, tbk, bk, kt_t, "pqk"))):
                    nc.sync.dma_start(wt[:], wtr_src)
                    nc.sync.dma_start(tb[:], tb_src)
                    for lh in range(LH):
                        xs = []
                        for e in range(ECH):
                            x = xp.tile([128, 1024], bf16, tag="x", bufs=16,
                                        name=f"x{ti}_{lh}_{e}")
                            dma_in(x[:], src_r[:, e, lh * 1024:(lh + 1) * 1024])
                            xs.append(x)
                        for half in range(2):
                            lt = lh * 2 + half
                            hs = slice(half * 512, (half + 1) * 512)
                            pp = [psp.tile([128, 512], f32, tag=ptag, bufs=4,
                                           name=f"p{ti}_{lt}_{m}")
                                  for m in range(2)]
                            for e in range(ECH):
                                for m in range(2):
                                    nc.tensor.matmul(
                                        pp[m][:],
                                        wt[:, e, m * 128:(m + 1) * 128],
                                        xs[e][:, hs],
                                        start=(e == 0), stop=(e == ECH - 1))
                            for m in range(2):
                                nc.vector.tensor_scalar_add(
                                    dst[m][:, lt * 512:(lt + 1) * 512],
                                    pp[m][:], tb[:, m, :])

                # ---- V: direct [l, o] output: x chunk stationary, W_V
                # moving; 4 live banks per l-tile ----
                nc.sync.dma_start(twv[:], wvr)
                for lh in range(LH):
                    xs = []
                    for e in range(ECH):
                        x = xp.tile([128, 1024], bf16, tag="x", bufs=16,
                                    name=f"xv_{lh}_{e}")
                        dma_in(x[:], vTr[:, e, lh * 1024:(lh + 1) * 1024])
                        xs.append(x)
                    for half in range(2):
                        lt = lh * 2 + half
                        pv = [psp.tile([128, 512], f32, tag="pv", bufs=4,
                                       name=f"pv{lt}_{j}")
                              for j in range(4)]
                        for e in range(ECH):
                            for j in range(4):
                                nc.tensor.matmul(
                                    pv[j][:, 0:OC],
                                    xs[e][:, half * 512 + j * 128:
                                          half * 512 + (j + 1) * 128],
                                    twv[:, e, :],
                                    start=(e == 0), stop=(e == ECH - 1))
                        for j in range(4):
                            lb = lt * 4 + j
                            nc.vector.tensor_copy(
                                v_t[lb][:, :, 0:D],
                                pv[j][:, 0:OC].rearrange("p (h d) -> p h d", d=D))
                            nc.vector.memset(v_t[lb][:, :, D:D + 1], 1.0)

            nc.sync.dma_start(two[:], wor)

            # ========== phase 2+3: attention with interleaved W_O ==========
            # Stage = GRP consecutive (head, kc) units of one (qt, pair).
            # Units alternate the pair's two heads (PE row offsets 0/64)
            # so adjacent score matmuls run in disjoint PE row-groups.
            stages = []
            for qt in range(LT):
                for pr in range(2):
                    if PAIR_INTERLEAVE:
                        units = [(pr * 2 + (u % 2), u // 2)
                                 for u in range(2 * KC)]
                    else:
                        units = [(pr * 2 + (u // KC), u % KC)
                                 for u in range(2 * KC)]
                    for s0 in range(0, 2 * KC, GRP):
                        stages.append((qt, pr, units[s0:s0 + GRP]))

            with (
                tc.tile_pool(name="ps_st", bufs=2, space="PSUM") as pst,
                tc.tile_pool(name="ps_o", bufs=2, space="PSUM") as pop,
            ):
                st_t = [None] * len(stages)
                po_t = {}
                deferred = []

                def flush_norm():
                    # broadcast+scale run well after the p_o bank was
                    # released and after the W_O chunk's ty copies, so the
                    # vector queue at a boundary is: pob/den copies, recips,
                    # tys, then these muls. Nothing reads ot for 11+ stages.
                    while deferred:
                        m, po, qs, pob, rec, nm = deferred.pop(0)
                        rec_b = npl.tile([D, 512], f32, tag="recb",
                                         name=f"recb{nm}")
                        nc.gpsimd.partition_broadcast(rec_b[:], rec[:])
                        nc.vector.tensor_mul(
                            ot_t[m][po:po + 64, qs], pob[:], rec_b[:])

                def emit_scores(s):
                    qt, pr, units = stages[s]
                    qs = slice(qt * 512, (qt + 1) * 512)
                    st = pst.tile([128, GRP, 512], f32, tag="st", name=f"st{s}")
                    st_t[s] = st
                    for j, (h, kc) in enumerate(units):
                        m, po = h // 2, (h % 2) * 64
                        nc.tensor.matmul(
                            st[:, j, :],
                            kt_t[m][po:po + 64, kc * 128:(kc + 1) * 128],
                            qt_t[m][po:po + 64, qs],
                            start=True, stop=True)

                def emit_act_pv(s):
                    qt, pr, units = stages[s]
                    qs = slice(qt * 512, (qt + 1) * 512)
                    g = len(units)
                    st = st_t[s]
                    et = ep.tile([128, GRP, 512], bf16, tag="et", name=f"et{s}")
                    nc.scalar.activation(et[:, 0:g, :], st[:, 0:g, :], Exp,
                                         scale=0.125)
                    for j, (h, kc) in enumerate(units):
                        if kc == 0:
                            po_t[(qt, h)] = pop.tile([128, 512], f32, tag="po",
                                                     name=f"po{qt}_{h}")
                        p_o = po_t[(qt, h)]
                        nc.tensor.matmul(
                            p_o[0:D + 1, :], v_t[kc][:, h, :], et[:, j, :],
                            start=(kc == 0), stop=(kc == KC - 1))
                        if kc == KC - 1:
                            # Release the p_o bank with two quick copies (a
                            # single 65-partition PSUM copy corrupts on HW;
                            # the split [0:64]+[64:65] form measures clean);
                            # den to SBUF also because approx recip does
                            # bitwise ops, invalid on the PSUM read path.
                            # The broadcast+scale are stashed and flushed
                            # after the boundary's W_O chunk (flush_norm).
                            m, po = h // 2, (h % 2) * 64
                            pob = npl.tile([D, 512], f32, tag="pob",
                                           name=f"pob{qt}_{h}")
                            nc.vector.tensor_copy(pob[:], p_o[0:D, :])
                            den = npl.tile([1, 512], f32, tag="den",
                                           name=f"den{qt}_{h}")
                            nc.vector.tensor_copy(den[:], p_o[D:D + 1, :])
                            rec = npl.tile([1, 512], f32, tag="rec",
                                           name=f"rec{qt}_{h}")
                            nc.vector.reciprocal_approx_fast(rec[:], den[:])
                            deferred.append(
                                (m, po, qs, pob, rec, f"{qt}_{h}"))

                def emit_phase3(qt, ecs, tail=False):
                    qs = slice(qt * 512, (qt + 1) * 512)
                    for ec in ecs:
                        py = pop.tile([128, 512], f32, tag="po",
                                      name=f"py{qt}_{ec}")
                        for oc in range(2):
                            nc.tensor.matmul(
                                py[:], two[:, oc, ec * 128:(ec + 1) * 128],
                                ot_t[oc][:, qs],
                                start=(oc == 0), stop=(oc == 1))
                        ty = ysp.tile([128, 512], bf16, tag="ty",
                                      name=f"ty{qt}_{ec}")
                        if tail and ec % 2 == 0:
                            # drain-only: ScalarE is idle after the last exp
                            # and can evacuate PSUM via a Copy activation
                            # (same table set as Exp - no reload)
                            nc.scalar.activation(ty[:], py[:], Copy,
                                                 scale=1.0)
                        else:
                            nc.vector.tensor_copy(ty[:], py[:])
                        nc.sync.dma_start(
                            yT[ec * 128:(ec + 1) * 128, qs], ty[:])

                # Emission: scores run 2 stages ahead (st double-buffer
                # permits it: scores(s+2) reuses the slot exp(s) just read)
                # so ScalarE never starves on a fresh score tile. The W_O
                # projection for a finished q-tile is emitted in 4-e-chunk
                # half-lumps at every pair boundary, where the po-tag slot
                # round-robin has a free window between the old pair's
                # normalize and the new pair's PV start.
                # Chunk scheduling: a qt's chunks become eligible one
                # boundary AFTER its last normalize (pop-before-append), and
                # each eligible chunk is emitted one stage past its boundary
                # (hold), BEFORE that stage's PVs: the py matmuls then wait
                # only on the pob copies (~1us), not the full normalize
                # chain, and the next score tiles aren't trapped behind
                # them, so ScalarE's 2-stage lookahead bridges the bubble.
                per_pair = (2 * KC + GRP - 1) // GRP
                pending = []
                hold = []
                emit_scores(0)
                emit_scores(1)
                for s in range(len(stages)):
                    if hold:
                        emit_phase3(*hold.pop(0))
                        flush_norm()
                    emit_act_pv(s)
                    if s + 2 < len(stages):
                        emit_scores(s + 2)
                    if (s + 1) % per_pair == 0:
                        if pending:
                            hold.append(pending.pop(0))
                        else:
                            flush_norm()
                        qt, pr, _ = stages[s]
                        if pr == 1:
                            pending.append((qt, range(0, 4)))
                            pending.append((qt, range(4, ECH)))
                flush_norm()
                while hold:
                    emit_phase3(*hold.pop(0), tail=True)
                while pending:
                    emit_phase3(*pending.pop(0), tail=True)

    nc.compile()
    return nc


def _get_nc():
    if "nc" not in _CACHE:
        _CACHE["nc"] = _build()
    return _CACHE["nc"]


def _make_in_maps(inputs):
    bf = ml_dtypes.bfloat16
    q = np.asarray(inputs["query"], dtype=np.float32)
    k = np.asarray(inputs["key"], dtype=np.float32)
    v = np.asarray(inputs["value"], dtype=np.float32)
    WQ = np.asarray(inputs["W_Query"], dtype=np.float32)
    WK = np.asarray(inputs["W_Key"], dtype=np.float32)
    WV = np.asarray(inputs["W_Value"], dtype=np.float32)
    WO = np.asarray(inputs["W_Output"], dtype=np.float32)
    BQ = np.asarray(inputs["B_Query"], dtype=np.float32)
    BK = np.asarray(inputs["B_Key"], dtype=np.float32)

    qTb = [np.ascontiguousarray(q[b].T.astype(bf)) for b in range(B)]
    kTb = [np.ascontiguousarray(k[b].T.astype(bf)) for b in range(B)]
    vTb = [np.ascontiguousarray(v[b].T.astype(bf)) for b in range(B)]

    def swiz_w(W):  # [E, OC] -> [128, ECH*OC]
        return np.ascontiguousarray(
            W.reshape(ECH, 128, OC).transpose(1, 0, 2).reshape(128, ECH * OC)
        ).astype(bf)

    in_maps = []
    for c in range(8):
        b, g = c // 4, c % 4
        sl = slice(OC * g, OC * (g + 1))
        wo_sw = np.ascontiguousarray(
            WO[sl, :].reshape(2, 128, E).transpose(1, 0, 2).reshape(128, 2 * E)
        ).astype(bf)
        in_maps.append({
            "qT": qTb[b],
            "kT": kTb[b],
            "vT": vTb[b],
            "wq": swiz_w(WQ[:, sl]),
            "wk": swiz_w(WK[:, sl]),
            "wv": swiz_w(WV[:, sl]),
            "wo": wo_sw,
            "bq": np.ascontiguousarray(
                BQ[sl].reshape(2, 128).T.reshape(128, 2, 1)),
            "bk": np.ascontiguousarray(
                BK[sl].reshape(2, 128).T.reshape(128, 2, 1)),
        })
    return in_maps


def _combine(results, inputs):
    WO = np.asarray(inputs["W_Output"], dtype=np.float32)
    BV = np.asarray(inputs["B_Value"], dtype=np.float32)
    BO = np.asarray(inputs["B_Output"], dtype=np.float32)
    out = np.zeros((B, L, E), dtype=np.float32)
    for c in range(8):
        out[c // 4] += results[c]["yT"].T.astype(np.float32)
    out += (BV @ WO + BO)[None, None, :]
    return out


def kernel(**inputs):
    from concourse.bass_utils import run_bass_kernel_spmd

    nc = _get_nc()
    in_maps = _make_in_maps(inputs)
    res = run_bass_kernel_spmd(nc, in_maps, list(range(8)))
    return _combine(res.results, inputs)
